# revision 39
# baseline (speedup 1.0000x reference)
"""NodeContrastiveLoss on 8 Trainium2 NeuronCores (Bass/Tile).

loss = mean_i[ -(z1n_i . z2n_i)/tau
               + log( sum_j exp((z1n_i . z2n_j)/tau)
                    + sum_{j!=i} exp((z1n_i . z1n_j)/tau) ) ]

The Scalar engine's exp stream is the bound (1 elem/lane/cycle @1.2GHz).
This version cuts exp work 22% below the naive split by exploiting the
symmetry of z1n@z1n.T: each unordered block pair {a,b} of the z1-z1
similarity is computed once, in quadrants, so every core's queries are its
OWN 2048 rows and total keys per query row shrink from 32768 to 25600:
  - all 16384 z2 keys           (phase 1, 8 chunks of 2048)
  - the 2048 own-block z1 keys  (phase 1 diag chunk, self term subtracted)
  - 7168 foreign z1 keys        (phase 2: for each peer p, one quadrant
    half per query half, so each z1-z1 off-diag block is computed exactly
    once somewhere)
Row sums come free via the exp accum_out; the transposed contribution of
each off-diag block (column sums) is computed by cheap PE ones-matmuls
(m=32 strips via tile_position) over the bf16 exp block that the Scalar
engine writes to SBUF, accumulated in a dedicated PSUM bank and exported.
The host combines row/col partials across cores and takes log + mean
(device can't: each row's sum needs cross-core parts).

PSUM: phase 1 uses 2x4-bank buffers (2048-key chunks); phase 2 (scoped
pools after release) 2x3-bank buffers (1536-key chunks) + 2x1 colsum bank.
"""

import os
import numpy as np

N, D = 16384, 128
TAU = 0.07
NCORES = 8
NQ = N // NCORES          # 2048 query rows per core
P = 128
QT = NQ // P              # 16 query tiles per core
H = NQ // 2               # 1024: quadrant half
GROUP = 16                # row tiles per staging group (2048 rows)
FD1 = 2048                # phase-1 chunk keys (4 PSUM banks)
FD2 = 1536                # phase-2 max chunk keys (3 PSUM banks)
NCC1 = 9                  # 8 z2 chunks + 1 diag chunk per qtile
CC2 = [1536, 1536, 1536, 1536, 1024]   # phase-2 chunk sizes (sum 7168)
FKEYS = 7168              # foreign keys per quadrant half
NCC = NCC1 + len(CC2)     # 14 accum slots per qtile
SUB = 512                 # matmul moving free dim (one PSUM bank)

_CACHE = {}


def _split_excess_waits(nc, mybir):
    """walrus in this env supports 1 sync-wait per instruction (2 for
    EventSemaphore); move excess waits onto injected same-engine NoOps."""
    n = 0
    for f in nc.m.functions:
        for bb in f.blocks:
            new_insts = None
            for idx, inst in enumerate(bb.instructions):
                si = getattr(inst, "sync_info", None)
                waits = list(si.on_wait) if si is not None and si.on_wait else []
                cap = 2 if getattr(inst, "opcode", None) == "EventSemaphore" else 1
                if len(waits) <= cap:
                    if new_insts is not None:
                        new_insts.append(inst)
                    continue
                if new_insts is None:
                    new_insts = list(bb.instructions[:idx])
                keep, excess = waits[-cap:], waits[:-cap]
                for w in excess:
                    n += 1
                    nop = mybir.InstNoOp(name=f"I-wsplit-{n}-{inst.name}", ins=[], outs=[])
                    nop.engine = inst.engine
                    nop.sync_info = mybir.SyncInfo(on_wait=[w], on_update=[])
                    new_insts.append(nop)
                si.on_wait = keep
                new_insts.append(inst)
            if new_insts is not None:
                bb.instructions = new_insts
    return n


def _build_nc():
    from contextlib import ExitStack

    import concourse.bass as bass
    import concourse.tile as tile
    from concourse import mybir

    F32 = mybir.dt.float32
    BF16 = mybir.dt.bfloat16
    AF = mybir.ActivationFunctionType
    ALU = mybir.AluOpType
    AX = mybir.AxisListType

    nc = bass.Bass("TRN2", target_bir_lowering=False, debug=False)
    # All row-tensors arrive HOST-PRE-TRANSPOSED to [128, rows] layout
    # (arr[p, t*128+d] = rows[t*128+p, d]) so every load is one contiguous
    # 8KB-per-partition DMA descriptor instead of 512B-row gathers (the
    # strided version made the first load ~13us and kept DMA rings busy
    # with descriptor overhead all run).
    z2 = nc.declare_dram_parameter("z2", [P, N], F32, isOutput=False).ap()
    z1q = nc.declare_dram_parameter("z1q", [P, NQ], F32, isOutput=False).ap()
    z2q = nc.declare_dram_parameter("z2q", [P, NQ], F32, isOutput=False).ap()
    fk = nc.declare_dram_parameter("fk", [P, 2 * FKEYS], F32, isOutput=False).ap()
    ones32 = nc.declare_dram_parameter("ones32", [P, 32], F32, isOutput=False).ap()
    out = nc.declare_dram_parameter("out", [P, 2 * QT], F32, isOutput=True).ap()
    ucols = nc.declare_dram_parameter("ucols", [2, FKEYS], F32, isOutput=True).ap()

    with tile.TileContext(nc) as tc, ExitStack() as ctx:
        persist = ctx.enter_context(tc.tile_pool(name="persist", bufs=1))
        stage_p = ctx.enter_context(tc.tile_pool(name="stage", bufs=2))
        norm_p = ctx.enter_context(tc.tile_pool(name="norms", bufs=2))
        nbg_p = ctx.enter_context(tc.tile_pool(name="nbg", bufs=2))
        work_p = ctx.enter_context(tc.tile_pool(name="work", bufs=4))
        exp_p = ctx.enter_context(tc.tile_pool(name="expb", bufs=6))
        ucst_p = ctx.enter_context(tc.tile_pool(name="ucst", bufs=2))

        I32 = mybir.dt.int32
        z2T = persist.tile([P, N], BF16, tag="z2T")
        fT = persist.tile([P, 2 * FKEYS], BF16, tag="fT")
        z1qT = persist.tile([P, NQ], BF16, tag="z1qT")
        z1qn = persist.tile([P, NQ], BF16, tag="z1qn")
        z2qn = persist.tile([P, NQ], BF16, tag="z2qn")
        pos_raw = persist.tile([P, QT], F32, tag="pos")
        d_raw = persist.tile([P, QT], F32, tag="draw")
        part = persist.tile([P, QT * NCC], F32, tag="part")
        part0 = persist.tile([P, 1], F32, tag="part0")
        onesb = persist.tile([P, 32], BF16, tag="onesb")

        def rsqrt_newton(ssq, ntiles):
            """r = 1/sqrt(ssq), all on DVE (keeps ACT free for the exp
            stream): fast-pow bitcast seed + 3 Newton steps.  HW-validated
            max rel err 2e-7."""
            xf = norm_p.tile([P, GROUP], F32, tag="xf")
            gi = norm_p.tile([P, GROUP], I32, tag="gi")
            r0 = norm_p.tile([P, GROUP], F32, tag="r0")
            t1 = norm_p.tile([P, GROUP], F32, tag="t1")
            nc.vector.tensor_copy(xf[:, :ntiles], ssq[:, :ntiles].bitcast(I32))
            nc.vector.tensor_scalar(
                out=xf[:, :ntiles], in0=xf[:, :ntiles], scalar1=-0.5,
                scalar2=190.5 * 8388608.0, op0=ALU.mult, op1=ALU.add)
            nc.vector.tensor_copy(gi[:, :ntiles], xf[:, :ntiles])
            nc.vector.tensor_copy(r0[:, :ntiles], gi[:, :ntiles].bitcast(F32))
            for _ in range(3):
                nc.vector.tensor_mul(t1[:, :ntiles], r0[:, :ntiles],
                                     r0[:, :ntiles])
                nc.vector.tensor_mul(t1[:, :ntiles], t1[:, :ntiles],
                                     ssq[:, :ntiles])
                nc.vector.tensor_scalar(
                    out=t1[:, :ntiles], in0=t1[:, :ntiles],
                    scalar1=-0.5, scalar2=1.5, op0=ALU.mult, op1=ALU.add,
                )
                nc.vector.tensor_mul(r0[:, :ntiles], r0[:, :ntiles],
                                     t1[:, :ntiles])
            return r0

        def load_raw(src, row0, ntiles):
            """DMA ntiles row tiles to staging; per-row sum of squares."""
            stage = stage_p.tile([P, GROUP, P], F32, tag="stage")
            nc.sync.dma_start(
                out=stage[:, :ntiles, :],
                in_=src[:, row0:row0 + ntiles * P].rearrange("p (t d) -> p t d", d=P),
            )
            ssq = norm_p.tile([P, GROUP], F32, tag="ssq")
            for t in range(ntiles):
                sq = work_p.tile([P, P], F32, tag="sq")
                nc.vector.scalar_tensor_tensor(
                    out=sq[:, :], in0=stage[:, t, :], scalar=1.0,
                    in1=stage[:, t, :], op0=ALU.bypass, op1=ALU.mult,
                    accum_out=ssq[:, t:t + 1],
                )
            return stage, ssq

        def load_group(src, row0, ntiles):
            stage, ssq = load_raw(src, row0, ntiles)
            return stage, rsqrt_newton(ssq, ntiles)

        def normalize_group(stage, r, ntiles):
            nbg = nbg_p.tile([P, GROUP * P], BF16, tag="nbg")
            for t in range(ntiles):
                nc.vector.tensor_scalar_mul(
                    nbg[:, t * P:(t + 1) * P], stage[:, t, :], r[:, t:t + 1])
            return nbg

        def transpose_group(nbg, dst_T, col0, ntiles, eng=None):
            dst3 = dst_T[:, col0:col0 + ntiles * P].rearrange(
                "p (t d) -> p t d", d=P)
            (eng or nc.sync).dma_start_transpose(dst3, nbg[:, :ntiles * P])

        def finish_keys(stg, ssq, dst_T, col0):
            r = rsqrt_newton(ssq, GROUP)
            nbg = normalize_group(stg, r, GROUP)
            transpose_group(nbg, dst_T, col0, GROUP)

        def stage_keys(src, row0, dst_T, col0):
            stg, ssq = load_raw(src, row0, GROUP)
            finish_keys(stg, ssq, dst_T, col0)

        def deferred_qprep():
            """z2q chain + pos + d: runs in engine slack under early exps."""
            stg, rq = load_group(z2q, 0, QT)
            for t in range(QT):
                nc.vector.tensor_scalar_mul(
                    z2qn[:, t * P:(t + 1) * P], stg[:, t, :], rq[:, t:t + 1])
            for t in range(QT):
                # d_raw[:, t] = sum_d bf16(z1n)^2 (matches the PE diag dot)
                sq = work_p.tile([P, P], F32, tag="dsq")
                nc.gpsimd.tensor_mul(sq[:, :], z1qn[:, t * P:(t + 1) * P],
                                     z1qn[:, t * P:(t + 1) * P])
                nc.vector.tensor_reduce(
                    out=d_raw[:, t:t + 1], in_=sq[:, :], axis=AX.X, op=ALU.add)
                # pos_raw[:, t] = sum_d bf16(z1n) * bf16(z2n) (f32 accum)
                mb = work_p.tile([P, P], F32, tag="mb")
                nc.gpsimd.tensor_mul(mb[:, :], z1qn[:, t * P:(t + 1) * P],
                                     z2qn[:, t * P:(t + 1) * P])
                nc.vector.tensor_reduce(
                    out=pos_raw[:, t:t + 1], in_=mb[:, :], axis=AX.X, op=ALU.add)

        # ---------------- prologue ----------------
        # Critical path to the first exp chunk is ONLY the own-block chain:
        # phase-1 cc0 is the diag chunk, whose keys are z1qT itself.  The
        # z2 g0 DMA+squares start concurrently, but g0's rsqrt Ln is NOT
        # emitted here — it would sit in the in-order ACT FIFO ahead of
        # cc0's already-runnable exps.  It is emitted under cc0 (qt==2).
        stage, ssqQ = load_raw(z1q, 0, QT)
        r = rsqrt_newton(ssqQ, QT)
        # normalize+transpose in halves: the first transpose (and the diag
        # chunk's first matmuls, which read z1qT progressively) start ~2us
        # earlier than with one monolithic 16-tile transpose
        for h in range(2):
            for t in range(h * QT // 2, (h + 1) * QT // 2):
                nc.vector.tensor_scalar_mul(
                    z1qn[:, t * P:(t + 1) * P], stage[:, t, :], r[:, t:t + 1])
            dst3 = z1qT[:, h * NQ // 2:(h + 1) * NQ // 2].rearrange(
                "p (t d) -> p t d", d=P)
            nc.sync.dma_start_transpose(dst3, z1qn[:, h * NQ // 2:
                                                   (h + 1) * NQ // 2])
        # z2 g0 load emitted after the whole own-block chain (its squares
        # still get slotted into the chain's bubbles by the scheduler, but
        # emitting it earlier or later measured worse)
        st0, sq0 = load_raw(z2, 0, GROUP)
        # ones (bf16) for the colsum matmuls
        onesf = persist.tile([P, 32], F32, tag="onesf")
        nc.sync.dma_start(out=onesf[:, :], in_=ones32[:, :])
        nc.vector.tensor_copy(onesb[:, :], onesf[:, :])

        # ---------------- phase 1: diag chunk + z2 keys (FD1=2048) --------
        # cc0 = diag (keys z1qT), cc 1..8 = z2 groups 0..7.  Staging chains
        # for later chunk-columns are emitted a few chunks into each column:
        # their rsqrt Ln/Exp land in the ACT FIFO behind already-runnable
        # exp chunks, so the in-order ACT queue never blocks on a staging
        # dependency (the baseline lost ~40us to this).
        with tc.tile_pool(name="ps1", bufs=2, space="PSUM") as ps1:
            for cc in range(NCC1):
                keysT, koff = (z1qT, 0) if cc == 0 else (z2T, (cc - 1) * FD1)
                for qt in range(QT):
                    if qt == 2 and cc == 0:
                        finish_keys(st0, sq0, z2T, 0)
                    if qt == 6 and cc <= 6:
                        stage_keys(z2, (cc + 1) * GROUP * P, z2T,
                                   (cc + 1) * GROUP * P)
                    if qt == 10 and cc == 0:
                        deferred_qprep()
                    if qt == 12 and 2 <= cc <= 8:
                        g = cc - 2
                        stage_keys(fk, g * GROUP * P, fT, g * GROUP * P)
                    # The very first chunk is split into two half chunks so
                    # the exp stream starts right after the FIRST half of
                    # the prologue transpose instead of the whole thing
                    # (~5us of ramp); its extra row-sum goes to part0.
                    subs = ((0, FD1 // 2, part0[:, 0:1]),
                            (FD1 // 2, FD1,
                             part[:, qt * NCC + cc:qt * NCC + cc + 1])) \
                        if cc == 0 and qt == 0 else \
                        ((0, FD1, part[:, qt * NCC + cc:qt * NCC + cc + 1]),)
                    for lo, hi, slot in subs:
                        ps = ps1.tile([P, FD1], F32, tag="ps")
                        for j in range(lo // SUB, hi // SUB):
                            nc.tensor.matmul(
                                ps[:, (j * SUB - lo):((j + 1) * SUB - lo)],
                                lhsT=z1qT[:, qt * P:(qt + 1) * P],
                                rhs=keysT[:, koff + j * SUB:
                                          koff + (j + 1) * SUB],
                                start=True, stop=True,
                            )
                        nc.scalar.activation(
                            ps[:, :hi - lo], ps[:, :hi - lo], AF.Exp,
                            bias=0.0, scale=1.0 / TAU, accum_out=slot,
                        )

        # ---------------- phase 2: foreign keys (symmetric off-diag) ------
        # chunk (c2, qh, k): queries = own qtile qh*8+k, keys = fT slice.
        # exp -> SBUF bf16 (rhs for colsum matmuls) + accum_out row sums.
        # colsum matmuls for batch (c2, qh) are emitted interleaved into the
        # NEXT batch's slots (deps already satisfied -> no PE stall), with
        # m=32 ones-lhsT strips accumulating into a 1-bank PSUM tile.
        def emit_colsum(prev, k):
            pcs, pebs, pfd = prev
            for s in range(pfd // SUB):
                nc.tensor.matmul(
                    pcs[32 * s:32 * s + 32, :],
                    lhsT=onesb[:, 0:32],
                    rhs=pebs[k][:, s * SUB:(s + 1) * SUB],
                    start=(k == 0), stop=(k == QT // 2 - 1),
                    tile_position=(0, 32 * s),
                )

        def emit_drain(prev, pqh, pc2):
            pcs, _, pfd = prev
            nst = pfd // SUB
            uc = ucst_p.tile([P, SUB], F32, tag="ucst")
            nc.vector.tensor_copy(uc[:, :], pcs[:, :])
            base = pc2 * FD2
            for s in range(nst):
                nc.sync.dma_start(
                    out=ucols[pqh:pqh + 1, base + s * SUB:base + (s + 1) * SUB],
                    in_=uc[32 * s:32 * s + 1, :],
                )

        with tc.tile_pool(name="ps2", bufs=2, space="PSUM") as ps2, \
                tc.tile_pool(name="cs", bufs=2, space="PSUM") as cs_p:
            prev = None      # (cs_tile, exp_tiles, fd) of previous batch
            prev_loc = None  # (qh, c2)
            for c2 in range(len(CC2)):
                fd = CC2[c2]
                for qh in range(2):
                    cs = cs_p.tile([P, SUB], F32, tag="cs")
                    ebs = []
                    for k in range(QT // 2):
                        qt = qh * (QT // 2) + k
                        koff = qh * FKEYS + c2 * FD2
                        ps = ps2.tile([P, FD2], F32, tag="ps")
                        for j in range(fd // SUB):
                            nc.tensor.matmul(
                                ps[:, j * SUB:(j + 1) * SUB],
                                lhsT=z1qT[:, qt * P:(qt + 1) * P],
                                rhs=fT[:, koff + j * SUB:koff + (j + 1) * SUB],
                                start=True, stop=True,
                            )
                        eb = exp_p.tile([P, FD2], BF16, tag="eb")
                        # previous batch's colsum matmuls: WAR-ordered before
                        # this ACT overwrites the rotated exp buffer
                        if prev is not None:
                            emit_colsum(prev, k)
                        nc.scalar.activation(
                            eb[:, :fd], ps[:, :fd], AF.Exp,
                            bias=0.0, scale=1.0 / TAU,
                            accum_out=part[:, qt * NCC + NCC1 + c2:
                                           qt * NCC + NCC1 + c2 + 1],
                        )
                        ebs.append(eb)
                    if prev is not None:
                        emit_drain(prev, *prev_loc)
                    prev, prev_loc = (cs, ebs, fd), (qh, c2)
            # last batch's colsums + drain (PE/DMA tail under the epilogue)
            for k in range(QT // 2):
                emit_colsum(prev, k)
            emit_drain(prev, *prev_loc)

        # ---------------- epilogue: per-row partial sums ----------------
        S_own = work_p.tile([P, QT], F32, tag="sown")
        for qt in range(QT):
            nc.vector.tensor_reduce(
                out=S_own[:, qt:qt + 1],
                in_=part[:, qt * NCC:(qt + 1) * NCC],
                axis=AX.X, op=ALU.add,
            )
        # first half of the split first chunk
        nc.vector.tensor_add(S_own[:, 0:1], S_own[:, 0:1], part0[:, 0:1])
        exp_d = work_p.tile([P, QT], F32, tag="expd")
        nc.scalar.activation(exp_d[:, :], d_raw[:, :], AF.Exp,
                             bias=0.0, scale=1.0 / TAU)
        res = work_p.tile([P, 2 * QT], F32, tag="res")
        nc.vector.tensor_sub(res[:, 0:QT], S_own[:, :], exp_d[:, :])
        nc.vector.tensor_copy(res[:, QT:2 * QT], pos_raw[:, :])
        nc.sync.dma_start(out=out[:, :], in_=res[:, :])

    _split_excess_waits(nc, mybir)
    return nc


def _get_nc():
    if "nc" not in _CACHE:
        _CACHE["nc"] = _build_nc()
    return _CACHE["nc"]


def _foreign_rows(c):
    """Per-core foreign key row indices: [qh, u] -> H rows of z1."""
    rows = []
    for qh in range(2):
        for d in range(1, NCORES):
            p = (c + d) % NCORES
            half = qh if c < p else 1 - qh
            off = p * NQ + half * H
            rows.append(np.arange(off, off + H))
    return np.concatenate(rows)


def kernel(z1, z2):
    from concourse.bass_utils import run_bass_kernel_spmd

    z1 = np.ascontiguousarray(np.asarray(z1, dtype=np.float32))
    z2 = np.ascontiguousarray(np.asarray(z2, dtype=np.float32))
    assert z1.shape == (N, D) and z2.shape == (N, D)

    nc = _get_nc()
    ones = np.ones((P, 32), dtype=np.float32)

    def to_pt(a):
        """[rows, 128] -> [128, rows] tile-transposed: out[p, t*128+d] =
        a[t*128+p, d], matching the kernel's staging layout."""
        T = a.shape[0] // P
        return np.ascontiguousarray(
            a.reshape(T, P, D).transpose(1, 0, 2).reshape(P, T * D))

    z2_pt = to_pt(z2)
    in_maps = [
        {
            "z2": z2_pt,
            "z1q": to_pt(z1[c * NQ:(c + 1) * NQ]),
            "z2q": to_pt(z2[c * NQ:(c + 1) * NQ]),
            "fk": to_pt(z1[_foreign_rows(c)]),
            "ones32": ones,
        }
        for c in range(NCORES)
    ]
    trace = bool(int(os.environ.get("TRNLOSS_TRACE", "0")))
    res = run_bass_kernel_spmd(nc, in_maps, core_ids=list(range(NCORES)), trace=trace)
    if trace:
        _CACHE["exec_time_ns"] = res.exec_time_ns
        print(f"HW exec time: {res.exec_time_ns} ns")

    S = np.zeros(N, dtype=np.float64)
    pos = np.zeros(N, dtype=np.float64)
    for c in range(NCORES):
        o = res.results[c]["out"].astype(np.float64)      # [P, 2*QT]
        S[c * NQ:(c + 1) * NQ] += o[:, 0:QT].T.reshape(-1)
        pos[c * NQ:(c + 1) * NQ] = o[:, QT:2 * QT].T.reshape(-1)
        uc = res.results[c]["ucols"].astype(np.float64)   # [2, FKEYS]
        for qh in range(2):
            for u in range(NCORES - 1):
                d = u + 1
                p = (c + d) % NCORES
                half = qh if c < p else 1 - qh
                off = p * NQ + half * H
                S[off:off + H] += uc[qh, u * H:(u + 1) * H]
    loss = np.mean(np.log(S) - pos / TAU)
    return np.float32(loss)


# revision 40
# speedup vs baseline: 1.0024x; 1.0024x over previous
"""NodeContrastiveLoss on 8 Trainium2 NeuronCores (Bass/Tile).

loss = mean_i[ -(z1n_i . z2n_i)/tau
               + log( sum_j exp((z1n_i . z2n_j)/tau)
                    + sum_{j!=i} exp((z1n_i . z1n_j)/tau) ) ]

The Scalar engine's exp stream is the bound (1 elem/lane/cycle @1.2GHz).
This version cuts exp work 22% below the naive split by exploiting the
symmetry of z1n@z1n.T: each unordered block pair {a,b} of the z1-z1
similarity is computed once, in quadrants, so every core's queries are its
OWN 2048 rows and total keys per query row shrink from 32768 to 25600:
  - all 16384 z2 keys           (phase 1, 8 chunks of 2048)
  - the 2048 own-block z1 keys  (phase 1 diag chunk, self term subtracted)
  - 7168 foreign z1 keys        (phase 2: for each peer p, one quadrant
    half per query half, so each z1-z1 off-diag block is computed exactly
    once somewhere)
Row sums come free via the exp accum_out; the transposed contribution of
each off-diag block (column sums) is computed by cheap PE ones-matmuls
(m=32 strips via tile_position) over the bf16 exp block that the Scalar
engine writes to SBUF, accumulated in a dedicated PSUM bank and exported.
The host combines row/col partials across cores and takes log + mean
(device can't: each row's sum needs cross-core parts).

PSUM: phase 1 uses 2x4-bank buffers (2048-key chunks); phase 2 (scoped
pools after release) 2x3-bank buffers (1536-key chunks) + 2x1 colsum bank.
"""

import os
import numpy as np

N, D = 16384, 128
TAU = 0.07
NCORES = 8
NQ = N // NCORES          # 2048 query rows per core
P = 128
QT = NQ // P              # 16 query tiles per core
H = NQ // 2               # 1024: quadrant half
GROUP = 16                # row tiles per staging group (2048 rows)
FD1 = 2048                # phase-1 chunk keys (4 PSUM banks)
FD2 = 1536                # phase-2 max chunk keys (3 PSUM banks)
NCC1 = 9                  # 8 z2 chunks + 1 diag chunk per qtile
CC2 = [1536, 1536, 1536, 1536, 1024]   # phase-2 chunk sizes (sum 7168)
FKEYS = 7168              # foreign keys per quadrant half
NCC = NCC1 + len(CC2)     # 14 accum slots per qtile
SUB = 512                 # matmul moving free dim (one PSUM bank)

_CACHE = {}


def _split_excess_waits(nc, mybir):
    """walrus in this env supports 1 sync-wait per instruction (2 for
    EventSemaphore); move excess waits onto injected same-engine NoOps."""
    n = 0
    for f in nc.m.functions:
        for bb in f.blocks:
            new_insts = None
            for idx, inst in enumerate(bb.instructions):
                si = getattr(inst, "sync_info", None)
                waits = list(si.on_wait) if si is not None and si.on_wait else []
                cap = 2 if getattr(inst, "opcode", None) == "EventSemaphore" else 1
                if len(waits) <= cap:
                    if new_insts is not None:
                        new_insts.append(inst)
                    continue
                if new_insts is None:
                    new_insts = list(bb.instructions[:idx])
                keep, excess = waits[-cap:], waits[:-cap]
                for w in excess:
                    n += 1
                    nop = mybir.InstNoOp(name=f"I-wsplit-{n}-{inst.name}", ins=[], outs=[])
                    nop.engine = inst.engine
                    nop.sync_info = mybir.SyncInfo(on_wait=[w], on_update=[])
                    new_insts.append(nop)
                si.on_wait = keep
                new_insts.append(inst)
            if new_insts is not None:
                bb.instructions = new_insts
    return n


def _build_nc():
    from contextlib import ExitStack

    import concourse.bass as bass
    import concourse.tile as tile
    from concourse import mybir

    F32 = mybir.dt.float32
    BF16 = mybir.dt.bfloat16
    AF = mybir.ActivationFunctionType
    ALU = mybir.AluOpType
    AX = mybir.AxisListType

    nc = bass.Bass("TRN2", target_bir_lowering=False, debug=False)
    # All row-tensors arrive HOST-PRE-TRANSPOSED to [128, rows] layout
    # (arr[p, t*128+d] = rows[t*128+p, d]) so every load is one contiguous
    # 8KB-per-partition DMA descriptor instead of 512B-row gathers (the
    # strided version made the first load ~13us and kept DMA rings busy
    # with descriptor overhead all run).
    z2 = nc.declare_dram_parameter("z2", [P, N], F32, isOutput=False).ap()
    z1q = nc.declare_dram_parameter("z1q", [P, NQ], F32, isOutput=False).ap()
    z2q = nc.declare_dram_parameter("z2q", [P, NQ], F32, isOutput=False).ap()
    fk = nc.declare_dram_parameter("fk", [P, 2 * FKEYS], F32, isOutput=False).ap()
    ones32 = nc.declare_dram_parameter("ones32", [P, 32], F32, isOutput=False).ap()
    out = nc.declare_dram_parameter("out", [P, 2 * QT], F32, isOutput=True).ap()
    ucols = nc.declare_dram_parameter("ucols", [2, FKEYS], F32, isOutput=True).ap()

    with tile.TileContext(nc) as tc, ExitStack() as ctx:
        persist = ctx.enter_context(tc.tile_pool(name="persist", bufs=1))
        stage_p = ctx.enter_context(tc.tile_pool(name="stage", bufs=2))
        norm_p = ctx.enter_context(tc.tile_pool(name="norms", bufs=2))
        nbg_p = ctx.enter_context(tc.tile_pool(name="nbg", bufs=2))
        work_p = ctx.enter_context(tc.tile_pool(name="work", bufs=4))
        exp_p = ctx.enter_context(tc.tile_pool(name="expb", bufs=6))
        ucst_p = ctx.enter_context(tc.tile_pool(name="ucst", bufs=2))

        I32 = mybir.dt.int32
        z2T = persist.tile([P, N], BF16, tag="z2T")
        fT = persist.tile([P, 2 * FKEYS], BF16, tag="fT")
        z1qT = persist.tile([P, NQ], BF16, tag="z1qT")
        z1qn = persist.tile([P, NQ], BF16, tag="z1qn")
        z2qn = persist.tile([P, NQ], BF16, tag="z2qn")
        pos_raw = persist.tile([P, QT], F32, tag="pos")
        d_raw = persist.tile([P, QT], F32, tag="draw")
        part = persist.tile([P, QT * NCC], F32, tag="part")
        part0 = persist.tile([P, 1], F32, tag="part0")
        onesb = persist.tile([P, 32], BF16, tag="onesb")

        def rsqrt_newton(ssq, ntiles):
            """r = 1/sqrt(ssq), all on DVE (keeps ACT free for the exp
            stream): fast-pow bitcast seed + 3 Newton steps.  HW-validated
            max rel err 2e-7."""
            xf = norm_p.tile([P, GROUP], F32, tag="xf")
            gi = norm_p.tile([P, GROUP], I32, tag="gi")
            r0 = norm_p.tile([P, GROUP], F32, tag="r0")
            t1 = norm_p.tile([P, GROUP], F32, tag="t1")
            nc.vector.tensor_copy(xf[:, :ntiles], ssq[:, :ntiles].bitcast(I32))
            nc.vector.tensor_scalar(
                out=xf[:, :ntiles], in0=xf[:, :ntiles], scalar1=-0.5,
                scalar2=190.5 * 8388608.0, op0=ALU.mult, op1=ALU.add)
            nc.vector.tensor_copy(gi[:, :ntiles], xf[:, :ntiles])
            nc.vector.tensor_copy(r0[:, :ntiles], gi[:, :ntiles].bitcast(F32))
            for _ in range(3):
                nc.vector.tensor_mul(t1[:, :ntiles], r0[:, :ntiles],
                                     r0[:, :ntiles])
                nc.vector.tensor_mul(t1[:, :ntiles], t1[:, :ntiles],
                                     ssq[:, :ntiles])
                nc.vector.tensor_scalar(
                    out=t1[:, :ntiles], in0=t1[:, :ntiles],
                    scalar1=-0.5, scalar2=1.5, op0=ALU.mult, op1=ALU.add,
                )
                nc.vector.tensor_mul(r0[:, :ntiles], r0[:, :ntiles],
                                     t1[:, :ntiles])
            return r0

        def load_raw(src, row0, ntiles):
            """DMA ntiles row tiles to staging; per-row sum of squares."""
            stage = stage_p.tile([P, GROUP, P], F32, tag="stage")
            nc.sync.dma_start(
                out=stage[:, :ntiles, :],
                in_=src[:, row0:row0 + ntiles * P].rearrange("p (t d) -> p t d", d=P),
            )
            ssq = norm_p.tile([P, GROUP], F32, tag="ssq")
            for t in range(ntiles):
                sq = work_p.tile([P, P], F32, tag="sq")
                nc.vector.scalar_tensor_tensor(
                    out=sq[:, :], in0=stage[:, t, :], scalar=1.0,
                    in1=stage[:, t, :], op0=ALU.bypass, op1=ALU.mult,
                    accum_out=ssq[:, t:t + 1],
                )
            return stage, ssq

        def load_group(src, row0, ntiles):
            stage, ssq = load_raw(src, row0, ntiles)
            return stage, rsqrt_newton(ssq, ntiles)

        def normalize_group(stage, r, ntiles):
            nbg = nbg_p.tile([P, GROUP * P], BF16, tag="nbg")
            for t in range(ntiles):
                nc.vector.tensor_scalar_mul(
                    nbg[:, t * P:(t + 1) * P], stage[:, t, :], r[:, t:t + 1])
            return nbg

        def transpose_group(nbg, dst_T, col0, ntiles, eng=None):
            dst3 = dst_T[:, col0:col0 + ntiles * P].rearrange(
                "p (t d) -> p t d", d=P)
            (eng or nc.sync).dma_start_transpose(dst3, nbg[:, :ntiles * P])

        def finish_keys(stg, ssq, dst_T, col0):
            r = rsqrt_newton(ssq, GROUP)
            nbg = normalize_group(stg, r, GROUP)
            transpose_group(nbg, dst_T, col0, GROUP)

        def stage_keys(src, row0, dst_T, col0):
            stg, ssq = load_raw(src, row0, GROUP)
            finish_keys(stg, ssq, dst_T, col0)

        def deferred_qprep():
            """z2q chain + pos + d: runs in engine slack under early exps."""
            stg, rq = load_group(z2q, 0, QT)
            for t in range(QT):
                nc.vector.tensor_scalar_mul(
                    z2qn[:, t * P:(t + 1) * P], stg[:, t, :], rq[:, t:t + 1])
            for t in range(QT):
                # d_raw[:, t] = sum_d bf16(z1n)^2 (matches the PE diag dot)
                sq = work_p.tile([P, P], F32, tag="dsq")
                nc.gpsimd.tensor_mul(sq[:, :], z1qn[:, t * P:(t + 1) * P],
                                     z1qn[:, t * P:(t + 1) * P])
                nc.vector.tensor_reduce(
                    out=d_raw[:, t:t + 1], in_=sq[:, :], axis=AX.X, op=ALU.add)
                # pos_raw[:, t] = sum_d bf16(z1n) * bf16(z2n) (f32 accum)
                mb = work_p.tile([P, P], F32, tag="mb")
                nc.gpsimd.tensor_mul(mb[:, :], z1qn[:, t * P:(t + 1) * P],
                                     z2qn[:, t * P:(t + 1) * P])
                nc.vector.tensor_reduce(
                    out=pos_raw[:, t:t + 1], in_=mb[:, :], axis=AX.X, op=ALU.add)

        # ---------------- prologue ----------------
        # Critical path to the first exp chunk is ONLY the own-block chain:
        # phase-1 cc0 is the diag chunk, whose keys are z1qT itself.  The
        # z2 g0 DMA+squares start concurrently, but g0's rsqrt Ln is NOT
        # emitted here — it would sit in the in-order ACT FIFO ahead of
        # cc0's already-runnable exps.  It is emitted under cc0 (qt==2).
        stage, ssqQ = load_raw(z1q, 0, QT)
        r = rsqrt_newton(ssqQ, QT)
        # normalize+transpose in halves: the first transpose (and the diag
        # chunk's first matmuls, which read z1qT progressively) start ~2us
        # earlier than with one monolithic 16-tile transpose
        for h in range(2):
            for t in range(h * QT // 2, (h + 1) * QT // 2):
                nc.vector.tensor_scalar_mul(
                    z1qn[:, t * P:(t + 1) * P], stage[:, t, :], r[:, t:t + 1])
            dst3 = z1qT[:, h * NQ // 2:(h + 1) * NQ // 2].rearrange(
                "p (t d) -> p t d", d=P)
            nc.sync.dma_start_transpose(dst3, z1qn[:, h * NQ // 2:
                                                   (h + 1) * NQ // 2])
        # z2 g0 load emitted after the whole own-block chain (its squares
        # still get slotted into the chain's bubbles by the scheduler, but
        # emitting it earlier or later measured worse)
        st0, sq0 = load_raw(z2, 0, GROUP)
        # ones (bf16) for the colsum matmuls
        onesf = persist.tile([P, 32], F32, tag="onesf")
        nc.sync.dma_start(out=onesf[:, :], in_=ones32[:, :])
        nc.vector.tensor_copy(onesb[:, :], onesf[:, :])

        # ---------------- phase 1: diag chunk + z2 keys (FD1=2048) --------
        # cc0 = diag (keys z1qT), cc 1..8 = z2 groups 0..7.  Staging chains
        # for later chunk-columns are emitted a few chunks into each column:
        # their rsqrt Ln/Exp land in the ACT FIFO behind already-runnable
        # exp chunks, so the in-order ACT queue never blocks on a staging
        # dependency (the baseline lost ~40us to this).
        with tc.tile_pool(name="ps1", bufs=2, space="PSUM") as ps1:
            for cc in range(NCC1):
                keysT, koff = (z1qT, 0) if cc == 0 else (z2T, (cc - 1) * FD1)
                for qt in range(QT):
                    if qt == 2 and cc == 0:
                        finish_keys(st0, sq0, z2T, 0)
                    if qt == 6 and cc <= 6:
                        stage_keys(z2, (cc + 1) * GROUP * P, z2T,
                                   (cc + 1) * GROUP * P)
                    if qt == 10 and cc == 0:
                        deferred_qprep()
                    # fk staging at qt8 (qt4 for the last group) so the
                    # final foreign keysT lands with margin before phase 2
                    # instead of just-in-time
                    if qt == (4 if cc == 8 else 8) and 2 <= cc <= 8:
                        g = cc - 2
                        stage_keys(fk, g * GROUP * P, fT, g * GROUP * P)
                    # The very first chunk is split into two half chunks so
                    # the exp stream starts right after the FIRST half of
                    # the prologue transpose instead of the whole thing
                    # (~5us of ramp); its extra row-sum goes to part0.
                    subs = ((0, FD1 // 2, part0[:, 0:1]),
                            (FD1 // 2, FD1,
                             part[:, qt * NCC + cc:qt * NCC + cc + 1])) \
                        if cc == 0 and qt == 0 else \
                        ((0, FD1, part[:, qt * NCC + cc:qt * NCC + cc + 1]),)
                    for lo, hi, slot in subs:
                        ps = ps1.tile([P, FD1], F32, tag="ps")
                        for j in range(lo // SUB, hi // SUB):
                            nc.tensor.matmul(
                                ps[:, (j * SUB - lo):((j + 1) * SUB - lo)],
                                lhsT=z1qT[:, qt * P:(qt + 1) * P],
                                rhs=keysT[:, koff + j * SUB:
                                          koff + (j + 1) * SUB],
                                start=True, stop=True,
                            )
                        nc.scalar.activation(
                            ps[:, :hi - lo], ps[:, :hi - lo], AF.Exp,
                            bias=0.0, scale=1.0 / TAU, accum_out=slot,
                        )

        # ---------------- phase 2: foreign keys (symmetric off-diag) ------
        # chunk (c2, qh, k): queries = own qtile qh*8+k, keys = fT slice.
        # exp -> SBUF bf16 (rhs for colsum matmuls) + accum_out row sums.
        # colsum matmuls for batch (c2, qh) are emitted interleaved into the
        # NEXT batch's slots (deps already satisfied -> no PE stall), with
        # m=32 ones-lhsT strips accumulating into a 1-bank PSUM tile.
        def emit_colsum(prev, k):
            pcs, pebs, pfd = prev
            for s in range(pfd // SUB):
                nc.tensor.matmul(
                    pcs[32 * s:32 * s + 32, :],
                    lhsT=onesb[:, 0:32],
                    rhs=pebs[k][:, s * SUB:(s + 1) * SUB],
                    start=(k == 0), stop=(k == QT // 2 - 1),
                    tile_position=(0, 32 * s),
                )

        def emit_drain(prev, pqh, pc2):
            pcs, _, pfd = prev
            nst = pfd // SUB
            uc = ucst_p.tile([P, SUB], F32, tag="ucst")
            nc.vector.tensor_copy(uc[:, :], pcs[:, :])
            base = pc2 * FD2
            for s in range(nst):
                nc.sync.dma_start(
                    out=ucols[pqh:pqh + 1, base + s * SUB:base + (s + 1) * SUB],
                    in_=uc[32 * s:32 * s + 1, :],
                )

        with tc.tile_pool(name="ps2", bufs=2, space="PSUM") as ps2, \
                tc.tile_pool(name="cs", bufs=2, space="PSUM") as cs_p:
            prev = None      # (cs_tile, exp_tiles, fd) of previous batch
            prev_loc = None  # (qh, c2)
            for c2 in range(len(CC2)):
                fd = CC2[c2]
                for qh in range(2):
                    cs = cs_p.tile([P, SUB], F32, tag="cs")
                    ebs = []
                    for k in range(QT // 2):
                        qt = qh * (QT // 2) + k
                        koff = qh * FKEYS + c2 * FD2
                        ps = ps2.tile([P, FD2], F32, tag="ps")
                        for j in range(fd // SUB):
                            nc.tensor.matmul(
                                ps[:, j * SUB:(j + 1) * SUB],
                                lhsT=z1qT[:, qt * P:(qt + 1) * P],
                                rhs=fT[:, koff + j * SUB:koff + (j + 1) * SUB],
                                start=True, stop=True,
                            )
                        eb = exp_p.tile([P, FD2], BF16, tag="eb")
                        # previous batch's colsum matmuls: WAR-ordered before
                        # this ACT overwrites the rotated exp buffer
                        if prev is not None:
                            emit_colsum(prev, k)
                        nc.scalar.activation(
                            eb[:, :fd], ps[:, :fd], AF.Exp,
                            bias=0.0, scale=1.0 / TAU,
                            accum_out=part[:, qt * NCC + NCC1 + c2:
                                           qt * NCC + NCC1 + c2 + 1],
                        )
                        ebs.append(eb)
                    if prev is not None:
                        emit_drain(prev, *prev_loc)
                    prev, prev_loc = (cs, ebs, fd), (qh, c2)
            # last batch's colsums + drain (PE/DMA tail under the epilogue)
            for k in range(QT // 2):
                emit_colsum(prev, k)
            emit_drain(prev, *prev_loc)

        # ---------------- epilogue: per-row partial sums ----------------
        S_own = work_p.tile([P, QT], F32, tag="sown")
        for qt in range(QT):
            nc.vector.tensor_reduce(
                out=S_own[:, qt:qt + 1],
                in_=part[:, qt * NCC:(qt + 1) * NCC],
                axis=AX.X, op=ALU.add,
            )
        # first half of the split first chunk
        nc.vector.tensor_add(S_own[:, 0:1], S_own[:, 0:1], part0[:, 0:1])
        exp_d = work_p.tile([P, QT], F32, tag="expd")
        nc.scalar.activation(exp_d[:, :], d_raw[:, :], AF.Exp,
                             bias=0.0, scale=1.0 / TAU)
        res = work_p.tile([P, 2 * QT], F32, tag="res")
        nc.vector.tensor_sub(res[:, 0:QT], S_own[:, :], exp_d[:, :])
        nc.vector.tensor_copy(res[:, QT:2 * QT], pos_raw[:, :])
        nc.sync.dma_start(out=out[:, :], in_=res[:, :])

    _split_excess_waits(nc, mybir)
    return nc


def _get_nc():
    if "nc" not in _CACHE:
        _CACHE["nc"] = _build_nc()
    return _CACHE["nc"]


def _foreign_rows(c):
    """Per-core foreign key row indices: [qh, u] -> H rows of z1."""
    rows = []
    for qh in range(2):
        for d in range(1, NCORES):
            p = (c + d) % NCORES
            half = qh if c < p else 1 - qh
            off = p * NQ + half * H
            rows.append(np.arange(off, off + H))
    return np.concatenate(rows)


def kernel(z1, z2):
    from concourse.bass_utils import run_bass_kernel_spmd

    z1 = np.ascontiguousarray(np.asarray(z1, dtype=np.float32))
    z2 = np.ascontiguousarray(np.asarray(z2, dtype=np.float32))
    assert z1.shape == (N, D) and z2.shape == (N, D)

    nc = _get_nc()
    ones = np.ones((P, 32), dtype=np.float32)

    def to_pt(a):
        """[rows, 128] -> [128, rows] tile-transposed: out[p, t*128+d] =
        a[t*128+p, d], matching the kernel's staging layout."""
        T = a.shape[0] // P
        return np.ascontiguousarray(
            a.reshape(T, P, D).transpose(1, 0, 2).reshape(P, T * D))

    z2_pt = to_pt(z2)
    in_maps = [
        {
            "z2": z2_pt,
            "z1q": to_pt(z1[c * NQ:(c + 1) * NQ]),
            "z2q": to_pt(z2[c * NQ:(c + 1) * NQ]),
            "fk": to_pt(z1[_foreign_rows(c)]),
            "ones32": ones,
        }
        for c in range(NCORES)
    ]
    trace = bool(int(os.environ.get("TRNLOSS_TRACE", "0")))
    res = run_bass_kernel_spmd(nc, in_maps, core_ids=list(range(NCORES)), trace=trace)
    if trace:
        _CACHE["exec_time_ns"] = res.exec_time_ns
        print(f"HW exec time: {res.exec_time_ns} ns")

    S = np.zeros(N, dtype=np.float64)
    pos = np.zeros(N, dtype=np.float64)
    for c in range(NCORES):
        o = res.results[c]["out"].astype(np.float64)      # [P, 2*QT]
        S[c * NQ:(c + 1) * NQ] += o[:, 0:QT].T.reshape(-1)
        pos[c * NQ:(c + 1) * NQ] = o[:, QT:2 * QT].T.reshape(-1)
        uc = res.results[c]["ucols"].astype(np.float64)   # [2, FKEYS]
        for qh in range(2):
            for u in range(NCORES - 1):
                d = u + 1
                p = (c + d) % NCORES
                half = qh if c < p else 1 - qh
                off = p * NQ + half * H
                S[off:off + H] += uc[qh, u * H:(u + 1) * H]
    loss = np.mean(np.log(S) - pos / TAU)
    return np.float32(loss)
